# revision 5
# baseline (speedup 1.0000x reference)
"""Chamfer (AutoEncoder) loss on 8 Trainium2 NeuronCores.

Problem: predictions [16, 2048, 3], targets [16, 2048, 3] (float32).
loss = sum_b [ sum_i min_j ||x_bi - y_bj||^2 + sum_j min_i ||x_bi - y_bj||^2 ]

Strategy (v2: single-P dual-fold)
--------------------------------
Data-parallel over the batch: 16 batches / 8 cores = 2 jobs per core.
Unlike v1 (which computed P and P^T separately, 2x matmul + paired DVE
reads + ACT copies), v2 computes each batch's distance matrix ONCE and
extracts BOTH reductions from a single DVE pass per strip:

  - a custom DVE op (hand-edited uop program) reads the PSUM strip
    (Src0) paired with the running column-accumulator (Src1, SBUF) and
    in ONE instruction emits:
        out      = max(Src0, Src1)   -> new column accumulator
        accum_out= max-fold(Src0)    -> per-row max of the strip
    (operands are negated, so max == min of distances).
  - after 16 strips the accumulator holds the column maxes of -P;
    16 PE transposes + one DVE tensor_reduce fold it across the
    partition axis into per-column results.

This halves PE work, frees the Scalar engine entirely, and runs the
DVE at its exact-arithmetic floor (every P element enters the DVE
exactly once; HW probes showed no 2x/4x DVE perf modes engage on this
toolchain, all engines reduce at 1 col/cycle, and only the DVE can
compute max at all).

fp32 matmul runs in LOW_HIGH mode (~8x slower than bf16), so operands
are split hi/lo in bf16 and K-stacked (hi*hi + hi*lo + lo*hi, K=15);
PE time scales with output columns, not K. PE row-group rotation
(partition offsets 0/32/64) keeps three sub-array pipelines running.
"""

import ml_dtypes
import numpy as np

import concourse.dve_ops as dve_ops
import concourse.mybir as mybir
import concourse.tile as tile
from concourse import bacc
from concourse.bass_utils import run_bass_kernel_spmd
from concourse.dve_ops import DveOp
from concourse.dve_spec import Spec, Src0, Src1, _has_src1, lower, maxx, minn
from concourse.dve_table_gen import dve_ver_for  # noqa: F401  (ver sanity)
from concourse.dve_uop import (
    DelayInp,
    DveOpSpec,
    OutPath,
    OutSel,
)


def _fold_free(a):
    return np.max(
        a.astype(np.float32), axis=tuple(range(1, a.ndim))
    ).reshape(a.shape[0], 1)


def _register_op(name, spec, edit=None):
    """Register a custom DVE op; optionally hand-edit the lowered uops.

    The edited program is injected into dve_ops._COMPILE_CACHE so both
    the per-NEFF table generator and the instruction emitter use it
    (DveOp.compile checks the cache before re-lowering; a cache miss
    would re-lower the spec and fail the pinned-sha check loudly).
    """
    for existing in dve_ops.OPS:
        if existing.name == name:
            return existing
    row = dve_ops._CUSTOM_DVE_ROW_BASE + len(dve_ops.OPS)
    shas = {}
    compiled = {}
    for ver in ("v3", "v4"):
        try:
            uops = lower(spec, ver=ver)
        except Exception:
            continue
        if edit is not None:
            uops = edit(uops, ver)
        s = DveOpSpec(name=name, opcode=row, uops=uops, rd1_en=_has_src1(spec))
        s.validate(ver)
        shas[ver] = s.sha(ver)
        compiled[ver] = s
    op = DveOp(name, spec, subdim=False, uops_sha=shas)
    dve_ops.OPS.append(op)
    dve_ops._SUB_OPCODE_FOR_NAME[op.name] = row
    dve_ops.CUSTOM_DVE_SPECS[op.name] = op.spec
    for ver, s in compiled.items():
        dve_ops._COMPILE_CACHE[(name, ver)] = s
    assert max(dve_ops._SUB_OPCODE_FOR_NAME.values()) < 0x20
    return op


def _register_maxpair_fold():
    """out = max(Src0, Src1); accum_out = max-fold(Src0).

    The Spec DSL can only fold the body root, so the body is
    min(Src0, max(Src0, Src1)) == Src0 — lower() then naturally builds:
      dp[0]: MAX(Src0, Src1)      (the pair max)
      dp[1]: MIN(Src0, PREV)      (== Src0, the fold input)
      dp[2]: accumulator MAX(CURR, PREV)  -> accum_out = fold(Src0)
      out   = DELAY_0 (captured root == Src0)
    The hand edit reroutes `out` to the dp[0] pair max: capture it into
    free delay lane 3 at dp[1] and select DELAY_3 as the write source.
    """

    def edit(uops, ver):
        assert len(uops) == 2, f"expected seed+steady, got {len(uops)}"
        seed, steady = uops
        assert steady.require_inp0 == 1, "uop order changed"
        for u in uops:
            for dp in u.datapath_config:
                dp.delay[3] = DelayInp.PREV_DELAY
                dp.delay_enable[3] = 1
        # capture dp[0]'s ALU out (the pair max) into lane 3 at stage 1
        steady.datapath_config[1].delay[3] = DelayInp.PREV_ALU_OUT
        steady.out[OutPath.WR0_LO] = OutSel.DELAY_3
        return uops

    spec = Spec(
        body=minn(Src0, maxx(Src0, Src1)),
        accum=maxx,
        reference=lambda in0, in1, s0, s1, imm2: (
            np.maximum(in0.astype(np.float32), in1.astype(np.float32)),
            _fold_free(in0),
        ),
    )
    return _register_op("MAXPAIR_FOLD0_ANT", spec, edit)


def _register_copy_fold():
    """out = Src0 (accumulator init); accum_out = max-fold(Src0)."""
    spec = Spec(
        body=Src0,
        accum=maxx,
        reference=lambda in0, in1, s0, s1, imm2: (
            in0.astype(np.float32),
            _fold_free(in0),
        ),
    )
    return _register_op("COPY_FOLD0_ANT", spec)


MAXPAIR_FOLD = _register_maxpair_fold()
COPY_FOLD = _register_copy_fold()

B, N, M, D = 16, 2048, 2048, 3
N_CORES = 8
JOBS = B // N_CORES  # batches per core (2); one job per batch
ROW_TILES = N // 128  # 16
COL_CHUNK = 512
KCAT = 15  # [hi; hi; lo] x [hi; lo; hi]

_F32 = mybir.dt.float32
_BF16 = mybir.dt.bfloat16
_NP_BF16 = ml_dtypes.bfloat16

_cached_nc = None


def _build_nc():
    nc = bacc.Bacc("TRN2", target_bir_lowering=False, debug=False)
    lhs = nc.dram_tensor("lhs", [JOBS, 3, KCAT, N], _BF16, kind="ExternalInput")
    rhs = nc.dram_tensor("rhs", [JOBS, 3, KCAT, M], _BF16, kind="ExternalInput")
    ident = nc.dram_tensor("ident", [128, 128], _F32, kind="ExternalInput")
    # 2 cols per strip (lo/hi half folds; host maxes the pair)
    rowm = nc.dram_tensor(
        "rowm", [JOBS, 128, 2 * ROW_TILES], _F32, kind="ExternalOutput"
    )
    colm = nc.dram_tensor("colm", [JOBS, 128, ROW_TILES], _F32, kind="ExternalOutput")

    with tile.TileContext(nc) as tc:
        with (
            tc.tile_pool(name="inp", bufs=3) as inp_pool,
            tc.tile_pool(name="psum", bufs=2, space="PSUM") as psum_pool,
            tc.tile_pool(name="acc", bufs=2) as acc_pool,
            tc.tile_pool(name="res", bufs=2) as res_pool,
            tc.tile_pool(name="one", bufs=1) as one_pool,
        ):
            ident_sb = one_pool.tile([128, 128], _F32, tag="ident")
            nc.sync.dma_start(ident_sb[:], ident[:, :])
            for j in range(JOBS):
                lhs_sb = inp_pool.tile([128, N], _BF16, tag="lhs")
                rhs_sb = inp_pool.tile([128, M], _BF16, tag="rhs")
                # Operand replicas at partition offsets 0/32/64 for PE
                # row-group rotation. Job 0's loads gate the ramp: spread
                # across three engines' DMA queues.
                engines = (nc.sync, nc.scalar, nc.gpsimd) if j == 0 else (nc.sync,) * 3
                for a, g in enumerate((0, 32, 64)):
                    engines[a].dma_start(lhs_sb[g : g + KCAT, :], lhs[j, a])
                    engines[a].dma_start(rhs_sb[g : g + KCAT, :], rhs[j, a])

                rowm_sb = res_pool.tile([128, 2 * ROW_TILES], _F32, tag="rowm")
                colm_sb = res_pool.tile([128, ROW_TILES], _F32, tag="colm")
                acc_a = acc_pool.tile([128, M], _F32, tag="acc_a")
                acc_b = acc_pool.tile([128, M], _F32, tag="acc_b")

                H = M // 2  # DVE/PSUM access patterns must stay <= 2 banks
                for i in range(ROW_TILES):
                    lo_ps = psum_pool.tile([128, H], _F32, tag="lo")
                    hi_ps = psum_pool.tile([128, H], _F32, tag="hi")
                    li = slice(i * 128, (i + 1) * 128)
                    for c in range(4):
                        g = ((i * 4 + c) % 3) * 32
                        cs = slice(c * COL_CHUNK, (c + 1) * COL_CHUNK)
                        dst = lo_ps if c < 2 else hi_ps
                        ds = slice((c % 2) * COL_CHUNK, (c % 2 + 1) * COL_CHUNK)
                        nc.tensor.matmul(
                            dst[:, ds],
                            lhs_sb[g : g + KCAT, li],
                            rhs_sb[g : g + KCAT, cs],
                            start=True,
                            stop=True,
                        )
                    cur, prv = (acc_a, acc_b) if i % 2 == 0 else (acc_b, acc_a)
                    for h, ps in ((0, lo_ps), (1, hi_ps)):
                        hs = slice(h * H, (h + 1) * H)
                        rs = slice(2 * i + h, 2 * i + h + 1)
                        if i == 0:
                            nc.vector._custom_dve(
                                COPY_FOLD,
                                out=cur[:, hs],
                                in0=ps[:],
                                accum_out=rowm_sb[:, rs],
                            )
                        else:
                            nc.vector._custom_dve(
                                MAXPAIR_FOLD,
                                out=cur[:, hs],
                                in0=ps[:],
                                in1=prv[:, hs],
                                accum_out=rowm_sb[:, rs],
                            )
                final_acc = acc_a if (ROW_TILES - 1) % 2 == 0 else acc_b

                # Column maxes: 16 PE transposes of the accumulator into two
                # PSUM tiles [128, 8, 128], each folded over the innermost
                # (original partition) axis by one DVE tensor_reduce.
                for h in range(2):
                    tp = psum_pool.tile([128, 8, 128], _F32, tag="lo" if h == 0 else "hi")
                    for t in range(8):
                        k = h * 8 + t
                        nc.tensor.transpose(
                            tp[:, t, :],
                            final_acc[:, k * 128 : (k + 1) * 128],
                            ident_sb[:],
                        )
                    nc.vector.tensor_reduce(
                        colm_sb[:, h * 8 : (h + 1) * 8],
                        tp[:],
                        mybir.AxisListType.X,
                        mybir.AluOpType.max,
                    )

                nc.sync.dma_start(rowm[j], rowm_sb[:])
                nc.sync.dma_start(colm[j], colm_sb[:])
    nc.compile()
    return nc


def _get_nc():
    global _cached_nc
    if _cached_nc is None:
        _cached_nc = _build_nc()
    return _cached_nc


def _augment(a, b):
    """a: [n, 3], b: [m, 3] -> (lhsT [5, n], rhs [5, m]) float32.

    lhsT is negated so the device matmul yields -P.
    """
    n = a.shape[0]
    m = b.shape[0]
    lhsT = np.empty((5, n), dtype=np.float32)
    lhsT[0:3] = -a.T
    lhsT[3] = -(a * a).sum(axis=1)
    lhsT[4] = -1.0
    rhs = np.empty((5, m), dtype=np.float32)
    rhs[0:3] = -2.0 * b.T
    rhs[3] = 1.0
    rhs[4] = (b * b).sum(axis=1)
    return lhsT, rhs


def _split_cat(lhs, rhs):
    """fp32 [J, 5, n] operands -> K-stacked bf16 [J, 3, 15, n] hi/lo forms."""
    lh = lhs.astype(_NP_BF16)
    ll = (lhs - lh.astype(np.float32)).astype(_NP_BF16)
    rh = rhs.astype(_NP_BF16)
    rl = (rhs - rh.astype(np.float32)).astype(_NP_BF16)
    lcat = np.concatenate([lh, lh, ll], axis=1)
    rcat = np.concatenate([rh, rl, rh], axis=1)
    lrep = np.repeat(lcat[:, None, :, :], 3, axis=1)
    rrep = np.repeat(rcat[:, None, :, :], 3, axis=1)
    return np.ascontiguousarray(lrep), np.ascontiguousarray(rrep)


_IDENT = np.eye(128, dtype=np.float32)


def _in_maps(predictions, targets):
    in_maps = []
    for core in range(N_CORES):
        lhs = np.empty((JOBS, 5, N), dtype=np.float32)
        rhs = np.empty((JOBS, 5, M), dtype=np.float32)
        for j in range(JOBS):
            b = core * JOBS + j
            lhs[j], rhs[j] = _augment(predictions[b], targets[b])
        lcat, rcat = _split_cat(lhs, rhs)
        in_maps.append({"lhs": lcat, "rhs": rcat, "ident": _IDENT})
    return in_maps


def _host_reduce(results):
    """Sum per-core rowm/colm outputs into the final scalar loss.

    rowm holds separate lo/hi half folds per strip (cols 2i / 2i+1);
    the row max is the max of the pair.
    """
    total = 0.0
    for core in range(N_CORES):
        rowm = results[core]["rowm"].astype(np.float64)
        pairs = rowm.reshape(JOBS, 128, ROW_TILES, 2)
        total -= pairs.max(axis=-1).sum()
        total -= results[core]["colm"].astype(np.float64).sum()
    return np.float32(total)


def kernel(predictions, targets):
    predictions = np.asarray(predictions, dtype=np.float32)
    targets = np.asarray(targets, dtype=np.float32)

    nc = _get_nc()
    res = run_bass_kernel_spmd(
        nc, _in_maps(predictions, targets), core_ids=list(range(N_CORES))
    )
    return _host_reduce(res.results)
